# revision 14
# baseline (speedup 1.0000x reference)
"""Talking-heads attention on 8 Trainium2 NeuronCores.

Sharding: data-parallel over (batch b in 0..3) x (query half in 0..1) -> 8 cores.
Each core computes K/V for its full batch sequence (1024) and attention for its
512 query rows. No collectives.

Math notes (per core, all layouts transposed so contractions sit on partitions):
  - mix_pre (and SCALE) folded into Q per output-head g:
    qs_g[hd, i] = qT[hd, i] * scaleT[hd, g], scaleT = SCALE*mix_pre[h(hd), g]
    (host-precomputed), so dotsT_g[j, i] = sum_hd kT[hd, j] * qs_g[hd, i].
  - softmax over j (partitions) without max-subtraction (|dots| <~ 6, safe).
    The denominator never touches the PE: DVE tree-sums the 8 key chunks,
    GPSIMD partition_all_reduce sums across partitions (result broadcast to
    all partitions), DVE reciprocal + in-place scale of attnT.
  - mix_post folded into V: Vt_g[j, (g',d)] = mix_post[g, g'] * v[j, (g',d)];
    out2T[(g'd), i] += sum_j Vt_g[j, gd] * attnT_g[j, i] accumulated in PSUM
    over g (6 banks) while dots for g+2 stream (lag-2 software pipeline).
  - out = out2T.T @ Wout + bout, f32r matmuls (1 cyc/row), bias folded in as
    a K=1 accumulating matmul.
"""

import numpy as np

import concourse.bass as bass
import concourse.bass_isa as bass_isa
import concourse.mybir as mybir
import concourse.tile as tile
from concourse import bacc
from concourse.bass_utils import run_bass_kernel_spmd

P = 128
DIM = 768
SEQ = 1024
IQ = 512            # query rows per core
H = 12
DH = 64
NC6 = DIM // P      # 6 chunks of the 768 dim
JC8 = SEQ // P      # 8 chunks of the key dim
SCALE = DH ** -0.5
F32 = mybir.dt.float32
F32R = mybir.dt.float32r
BF16 = mybir.dt.bfloat16
EXP = mybir.ActivationFunctionType.Exp
ADD = mybir.AluOpType.add
MULT = mybir.AluOpType.mult

_CACHE = {}


def _build_nc():
    nc = bacc.Bacc("TRN2", target_bir_lowering=False, debug=False)

    xqT = nc.dram_tensor("xqT", [DIM, IQ], BF16, kind="ExternalInput")
    xkvT = nc.dram_tensor("xkvT", [DIM, SEQ], BF16, kind="ExternalInput")
    Wq = nc.dram_tensor("Wq", [DIM, DIM], BF16, kind="ExternalInput")
    Wk = nc.dram_tensor("Wk", [DIM, DIM], BF16, kind="ExternalInput")
    Wv = nc.dram_tensor("Wv", [DIM, DIM], BF16, kind="ExternalInput")
    Wout = nc.dram_tensor("Wout", [DIM, DIM], BF16, kind="ExternalInput")
    bout = nc.dram_tensor("bout", [1, DIM], BF16, kind="ExternalInput")
    scaleT_d = nc.dram_tensor("scaleT", [P, NC6 * H], F32, kind="ExternalInput")
    m2_d = nc.dram_tensor("m2", [1, H * H], F32, kind="ExternalInput")
    out = nc.dram_tensor("out", [IQ, DIM], F32, kind="ExternalOutput")

    r3 = lambda t: t.rearrange("(c p) e -> p c e", p=P)

    with tile.TileContext(nc) as tc:
        with tc.tile_pool(name="persist", bufs=1) as pp:
            # ---- persistent tiles ----
            qT = pp.tile([P, NC6, IQ], BF16)
            kT = pp.tile([P, NC6, SEQ], BF16)
            V = pp.tile([P, JC8, DIM], BF16)     # [j-part, jc, (g,d)]
            Wout_sb = pp.tile([P, NC6, DIM], BF16)
            o2_sb = pp.tile([P, NC6, IQ], BF16)  # out2T staged for out-proj
            scaleT = pp.tile([P, NC6 * H], F32)  # SCALE*mix_pre expanded
            m2_sb = pp.tile([1, H * H], F32)
            m2bc = pp.tile([P, H * H], F32)      # mix_post bcast to all parts
            bout_sb = pp.tile([1, DIM], BF16)
            ones_row = pp.tile([1, P], BF16)
            nc.gpsimd.memset(ones_row[:], 1.0)

            # ---- phase 1: input DMA (priority-ordered, chunked) + projections
            with (
                tc.tile_pool(name="pin", bufs=1) as pin,
                tc.tile_pool(name="pj", bufs=2, space="PSUM") as pj,
                tc.tile_pool(name="pjv", bufs=2, space="PSUM") as pjv,
            ):
                xqT_sb = pin.tile([P, NC6, IQ], BF16)
                Wq_sb = pin.tile([P, NC6, DIM], BF16)
                xkvT_sb = pin.tile([P, NC6, SEQ], BF16)
                Wk_sb = pin.tile([P, NC6, DIM], BF16)
                Wv_sb = pin.tile([P, NC6, DIM], BF16)
                # input DMA: one queue, one descriptor per tensor, in
                # consumption order — transfers pipeline behind the NEFF
                # startup window and the q projection.
                nc.sync.dma_start(Wq_sb[:], r3(Wq))
                nc.sync.dma_start(xqT_sb[:], r3(xqT))
                nc.sync.dma_start(xkvT_sb[:], r3(xkvT))
                nc.sync.dma_start(Wk_sb[:], r3(Wk))
                nc.sync.dma_start(Wv_sb[:], r3(Wv))
                nc.gpsimd.dma_start(scaleT[:], scaleT_d[:])
                nc.gpsimd.dma_start(m2_sb[:], m2_d[:])
                nc.gpsimd.dma_start(bout_sb[:], bout[:])
                nc.gpsimd.partition_broadcast(m2bc[:], m2_sb[:])

                # qT[e,i] = sum_f Wq[f,e] xqT[f,i]
                for ec in range(NC6):
                    ps = pj.tile([P, IQ], F32, tag="pjq")
                    for fc in range(NC6):
                        nc.tensor.matmul(
                            ps[:], Wq_sb[:, fc, ec * P : (ec + 1) * P],
                            xqT_sb[:, fc, :], start=(fc == 0), stop=(fc == NC6 - 1),
                        )
                    nc.scalar.copy(qT[:, ec, :], ps[:])

                # kT[e,j]
                for ec in range(NC6):
                    for jh in range(2):
                        ps = pj.tile([P, IQ], F32, tag="pjq")
                        for fc in range(NC6):
                            nc.tensor.matmul(
                                ps[:], Wk_sb[:, fc, ec * P : (ec + 1) * P],
                                xkvT_sb[:, fc, jh * IQ : (jh + 1) * IQ],
                                start=(fc == 0), stop=(fc == NC6 - 1),
                            )
                        nc.scalar.copy(kT[:, ec, jh * IQ : (jh + 1) * IQ], ps[:])

                # V[j, gd] = sum_f xkvT[f, j] Wv[f, gd]
                for jc in range(JC8):
                    psv = pjv.tile([P, DIM], F32, tag="pjv")
                    for ns, ne in ((0, 512), (512, DIM)):
                        for fc in range(NC6):
                            nc.tensor.matmul(
                                psv[:, ns:ne],
                                xkvT_sb[:, fc, jc * P : (jc + 1) * P],
                                Wv_sb[:, fc, ns:ne],
                                start=(fc == 0), stop=(fc == NC6 - 1),
                            )
                    nc.scalar.copy(V[:, jc, :], psv[:])

            # ---- phase 2: attention, lag-2 pipelined; softmax denom off-PE
            with (
                tc.tile_pool(name="acc", bufs=1, space="PSUM") as acc,
                tc.tile_pool(name="pds", bufs=2, space="PSUM") as pds,
                tc.tile_pool(name="attnp", bufs=3) as attnp,
                tc.tile_pool(name="qsp", bufs=2) as qsp,
                tc.tile_pool(name="vtp", bufs=3) as vtp,
                tc.tile_pool(name="sm1", bufs=1) as sm1,
                tc.tile_pool(name="sm2", bufs=2) as sm2,
            ):
                o2ps = [
                    acc.tile([P, IQ], F32, tag=f"o2_{s}", name=f"o2_{s}")
                    for s in range(NC6)
                ]
                s1 = sm1.tile([P, 4, IQ], BF16)
                s2 = sm1.tile([P, 2, IQ], BF16)

                attnTs, vts = {}, {}
                for it in range(H + 2):
                    if it == 1:
                        # Wout is only read by phase 3; issuing its DMA here
                        # keeps it out of the prologue bandwidth window.
                        nc.gpsimd.dma_start(Wout_sb[:], r3(Wout))
                    if it < H:
                        g = it
                        # qs(g) on DVE: per-partition scale fold (ACT is the
                        # exp bottleneck during pipeline fill)
                        qs = qsp.tile([P, NC6, IQ], BF16, tag="qs")
                        for c in range(NC6):
                            nc.vector.tensor_scalar_mul(
                                qs[:, c, :], qT[:, c, :],
                                scaleT[:, c * H + g : c * H + g + 1],
                            )
                        # Vt(g) on DVE: mix_post column fold per 64-col group
                        vt = vtp.tile([P, JC8, DIM], BF16, tag="vt")
                        vts[g] = vt
                        for gp in range(H):
                            nc.vector.tensor_scalar_mul(
                                vt[:, :, gp * DH : (gp + 1) * DH],
                                V[:, :, gp * DH : (gp + 1) * DH],
                                m2bc[:, g * H + gp : g * H + gp + 1],
                            )
                        # dots(g) on PE + exp on ACT
                        attnT = attnp.tile([P, JC8, IQ], BF16, tag="attnT")
                        attnTs[g] = attnT
                        for jc in range(JC8):
                            ds = pds.tile([P, IQ], F32, tag="ds")
                            for c in range(NC6):
                                nc.tensor.matmul(
                                    ds[:], kT[:, c, jc * P : (jc + 1) * P],
                                    qs[:, c, :],
                                    start=(c == 0), stop=(c == NC6 - 1),
                                )
                            nc.scalar.activation(attnT[:, jc, :], ds[:], EXP)
                        # softmax denominator: DVE jc-tree + GPSIMD allreduce
                        Ssum = sm2.tile([P, IQ], F32, tag="Ssum")
                        rR = sm2.tile([P, IQ], F32, tag="rR")
                        rRb = sm2.tile([P, IQ], BF16, tag="rRb")
                        nc.vector.tensor_tensor(
                            s1[:], attnT[:, 0:4, :], attnT[:, 4:8, :], ADD
                        )
                        nc.vector.tensor_tensor(
                            s2[:], s1[:, 0:2, :], s1[:, 2:4, :], ADD
                        )
                        nc.vector.tensor_tensor(
                            Ssum[:], s2[:, 0, :], s2[:, 1, :], ADD
                        )
                        nc.gpsimd.partition_all_reduce(
                            Ssum[:], Ssum[:], channels=P,
                            reduce_op=bass_isa.ReduceOp.add,
                        )
                        nc.vector.reciprocal_approx_fast(rR[:], Ssum[:])
                        nc.scalar.copy(rRb[:], rR[:])
                        nc.vector.tensor_tensor(
                            attnT[:], attnT[:],
                            rRb[:, None, :].to_broadcast((P, JC8, IQ)), MULT,
                        )
                    if it >= 2:
                        g2 = it - 2
                        for s in range(NC6):
                            for jc in range(JC8):
                                nc.tensor.matmul(
                                    o2ps[s][:],
                                    vts[g2][:, jc, s * P : (s + 1) * P],
                                    attnTs[g2][:, jc, :],
                                    start=(g2 == 0 and jc == 0),
                                    stop=(g2 == H - 1 and jc == JC8 - 1),
                                )
                        del attnTs[g2], vts[g2]

                for s in range(NC6):
                    if s % 2 == 0:
                        nc.vector.tensor_copy(o2_sb[:, s, :], o2ps[s][:])
                    else:
                        nc.scalar.copy(o2_sb[:, s, :], o2ps[s][:])

            # ---- phase 3: output projection (f32r) + bias via K=1 matmul ----
            with (
                tc.tile_pool(name="pj3", bufs=2, space="PSUM") as pj3,
                tc.tile_pool(name="ob", bufs=2) as ob,
            ):
                for isl in range(IQ // P):
                    fp = pj3.tile([P, DIM], F32, tag="fin")
                    for ns, ne in ((0, 512), (512, DIM)):
                        for c in range(NC6):
                            nc.tensor.matmul(
                                fp[:, ns:ne],
                                o2_sb[:, c, isl * P : (isl + 1) * P],
                                Wout_sb[:, c, ns:ne],
                                start=(c == 0), stop=False,
                            )
                        nc.tensor.matmul(
                            fp[:, ns:ne],
                            ones_row[:],
                            bout_sb[:, ns:ne],
                            start=False, stop=True,
                        )
                    osb = ob.tile([P, DIM], F32, tag="osb")
                    nc.scalar.copy(osb[:], fp[:])
                    nc.gpsimd.dma_start(out[isl * P : (isl + 1) * P, :], osb[:])

    nc.compile()
    return nc


def kernel(x, Wq, Wkv, mix_pre, mix_post, Wout, bout):
    x = np.asarray(x, dtype=np.float32)
    Wq = np.asarray(Wq, dtype=np.float32)
    Wkv = np.asarray(Wkv, dtype=np.float32)
    mix_pre = np.asarray(mix_pre, dtype=np.float32)
    mix_post = np.asarray(mix_post, dtype=np.float32)
    Wout = np.asarray(Wout, dtype=np.float32)
    bout = np.asarray(bout, dtype=np.float32)

    if "nc" not in _CACHE:
        _CACHE["nc"] = _build_nc()
    nc = _CACHE["nc"]

    import ml_dtypes
    bf = ml_dtypes.bfloat16
    Wk = np.ascontiguousarray(Wkv[:, :DIM]).astype(bf)
    Wv = np.ascontiguousarray(Wkv[:, DIM:]).astype(bf)
    # scaleT[p, c*H+g] = SCALE * mix_pre[(c*128+p)//64, g]
    SM = SCALE * mix_pre[np.arange(DIM) // DH, :]          # [768, 12]
    scaleT = np.ascontiguousarray(
        SM.reshape(NC6, P, H).transpose(1, 0, 2).reshape(P, NC6 * H)
    ).astype(np.float32)
    shared = {
        "Wq": Wq.astype(bf), "Wk": Wk, "Wv": Wv, "Wout": Wout.astype(bf),
        "bout": np.ascontiguousarray(bout.reshape(1, DIM)).astype(bf),
        "scaleT": scaleT,
        "m2": np.ascontiguousarray(mix_post.reshape(1, H * H)),
    }
    b_, n_, d_ = x.shape
    in_maps = []
    for c in range(8):
        b, half = c // 2, c % 2
        m = dict(shared)
        m["xqT"] = np.ascontiguousarray(x[b, half * IQ : (half + 1) * IQ, :].T).astype(bf)
        m["xkvT"] = np.ascontiguousarray(x[b].T).astype(bf)
        in_maps.append(m)

    res = run_bass_kernel_spmd(nc, in_maps, core_ids=list(range(8)))
    _CACHE["last_results"] = res
    _CACHE["last_in_maps"] = in_maps

    full = np.empty((b_, n_, d_), dtype=np.float32)
    for c in range(8):
        b, half = c // 2, c % 2
        full[b, half * IQ : (half + 1) * IQ, :] = res.results[c]["out"]
    return full


# revision 17
# speedup vs baseline: 1.1652x; 1.1652x over previous
"""Talking-heads attention on 8 Trainium2 NeuronCores.

Sharding: data-parallel over (batch b in 0..3) x (query half in 0..1) -> 8 cores.
Each core computes K/V for its full batch sequence (1024) and attention for its
512 query rows. No collectives.

Math notes (per core, all layouts transposed so contractions sit on partitions):
  - mix_pre (and SCALE) folded into Q per output-head g:
    qs_g[hd, i] = qT[hd, i] * scaleT[hd, g], scaleT = SCALE*mix_pre[h(hd), g]
    (host-precomputed), so dotsT_g[j, i] = sum_hd kT[hd, j] * qs_g[hd, i].
  - softmax over j (partitions) without max-subtraction (|dots| <~ 6, safe).
    The denominator never touches the PE: DVE tree-sums the 8 key chunks,
    GPSIMD partition_all_reduce sums across partitions (result broadcast to
    all partitions), DVE reciprocal + in-place scale of attnT.
  - mix_post folded into V: Vt_g[j, (g',d)] = mix_post[g, g'] * v[j, (g',d)];
    out2T[(g'd), i] += sum_j Vt_g[j, gd] * attnT_g[j, i] accumulated in PSUM
    over g (6 banks) while dots for g+2 stream (lag-2 software pipeline).
  - out = out2T.T @ Wout + bout, f32r matmuls (1 cyc/row), bias folded in as
    a K=1 accumulating matmul.
"""

import numpy as np

import concourse.bass as bass
import concourse.bass_isa as bass_isa
import concourse.mybir as mybir
import concourse.tile as tile
from concourse import bacc
from concourse.bass_utils import run_bass_kernel_spmd

P = 128
DIM = 768
SEQ = 1024
IQ = 512            # query rows per core
H = 12
DH = 64
NC6 = DIM // P      # 6 chunks of the 768 dim
JC8 = SEQ // P      # 8 chunks of the key dim
SCALE = DH ** -0.5
F32 = mybir.dt.float32
F32R = mybir.dt.float32r
BF16 = mybir.dt.bfloat16
EXP = mybir.ActivationFunctionType.Exp
ADD = mybir.AluOpType.add
MULT = mybir.AluOpType.mult

_CACHE = {}


def _build_nc():
    nc = bacc.Bacc("TRN2", target_bir_lowering=False, debug=False)

    xqT = nc.dram_tensor("xqT", [DIM, IQ], BF16, kind="ExternalInput")
    xkvT = nc.dram_tensor("xkvT", [DIM, SEQ], BF16, kind="ExternalInput")
    Wq = nc.dram_tensor("Wq", [DIM, DIM], BF16, kind="ExternalInput")
    Wk = nc.dram_tensor("Wk", [DIM, DIM], BF16, kind="ExternalInput")
    Wv = nc.dram_tensor("Wv", [DIM, DIM], BF16, kind="ExternalInput")
    Wout = nc.dram_tensor("Wout", [DIM, DIM], BF16, kind="ExternalInput")
    bout = nc.dram_tensor("bout", [1, DIM], BF16, kind="ExternalInput")
    scaleT_d = nc.dram_tensor("scaleT", [P, NC6 * H], F32, kind="ExternalInput")
    m2_d = nc.dram_tensor("m2", [1, H * H], F32, kind="ExternalInput")
    out = nc.dram_tensor("out", [IQ, DIM], F32, kind="ExternalOutput")

    r3 = lambda t: t.rearrange("(c p) e -> p c e", p=P)

    with tile.TileContext(nc) as tc:
        with tc.tile_pool(name="persist", bufs=1) as pp:
            # ---- persistent tiles ----
            qT = pp.tile([P, NC6, IQ], BF16)
            kT = pp.tile([P, NC6, SEQ], BF16)
            V = pp.tile([P, JC8, DIM], BF16)     # [j-part, jc, (g,d)]
            Wout_sb = pp.tile([P, NC6, DIM], BF16)
            o2_sb = pp.tile([P, NC6, IQ], BF16)  # out2T staged for out-proj
            scaleT = pp.tile([P, NC6 * H], F32)  # SCALE*mix_pre expanded
            m2_sb = pp.tile([1, H * H], F32)
            m2bc = pp.tile([P, H * H], F32)      # mix_post bcast to all parts
            bout_sb = pp.tile([1, DIM], BF16)
            ones_row = pp.tile([1, P], BF16)
            nc.gpsimd.memset(ones_row[:], 1.0)

            # ---- phase 1: input DMA (priority-ordered, chunked) + projections
            with (
                tc.tile_pool(name="pin", bufs=1) as pin,
                tc.tile_pool(name="pj", bufs=2, space="PSUM") as pj,
                tc.tile_pool(name="pjv", bufs=2, space="PSUM") as pjv,
            ):
                xqT_sb = pin.tile([P, NC6, IQ], BF16)
                Wq_sb = pin.tile([P, NC6, DIM], BF16)
                xkvT_sb = pin.tile([P, NC6, SEQ], BF16)
                Wk_sb = pin.tile([P, NC6, DIM], BF16)
                Wv_sb = pin.tile([P, NC6, DIM], BF16)
                # input DMA: one queue, one descriptor per tensor, in
                # consumption order — transfers pipeline behind the NEFF
                # startup window and the q projection.
                nc.sync.dma_start(Wq_sb[:], r3(Wq))
                nc.sync.dma_start(xqT_sb[:], r3(xqT))
                nc.sync.dma_start(xkvT_sb[:], r3(xkvT))
                nc.sync.dma_start(Wk_sb[:], r3(Wk))
                nc.sync.dma_start(Wv_sb[:], r3(Wv))
                nc.gpsimd.dma_start(scaleT[:], scaleT_d[:])
                nc.gpsimd.dma_start(m2_sb[:], m2_d[:])
                nc.gpsimd.dma_start(bout_sb[:], bout[:])
                nc.gpsimd.partition_broadcast(m2bc[:], m2_sb[:])

                # qT[e,i] = sum_f Wq[f,e] xqT[f,i]
                for ec in range(NC6):
                    ps = pj.tile([P, IQ], F32, tag="pjq")
                    for fc in range(NC6):
                        nc.tensor.matmul(
                            ps[:], Wq_sb[:, fc, ec * P : (ec + 1) * P],
                            xqT_sb[:, fc, :], start=(fc == 0), stop=(fc == NC6 - 1),
                        )
                    nc.scalar.copy(qT[:, ec, :], ps[:])

                # kT[e,j]
                for ec in range(NC6):
                    for jh in range(2):
                        ps = pj.tile([P, IQ], F32, tag="pjq")
                        for fc in range(NC6):
                            nc.tensor.matmul(
                                ps[:], Wk_sb[:, fc, ec * P : (ec + 1) * P],
                                xkvT_sb[:, fc, jh * IQ : (jh + 1) * IQ],
                                start=(fc == 0), stop=(fc == NC6 - 1),
                            )
                        nc.scalar.copy(kT[:, ec, jh * IQ : (jh + 1) * IQ], ps[:])

                # V[j, gd] = sum_f xkvT[f, j] Wv[f, gd]
                for jc in range(JC8):
                    psv = pjv.tile([P, DIM], F32, tag="pjv")
                    for ns, ne in ((0, 512), (512, DIM)):
                        for fc in range(NC6):
                            nc.tensor.matmul(
                                psv[:, ns:ne],
                                xkvT_sb[:, fc, jc * P : (jc + 1) * P],
                                Wv_sb[:, fc, ns:ne],
                                start=(fc == 0), stop=(fc == NC6 - 1),
                            )
                    nc.scalar.copy(V[:, jc, :], psv[:])

            # ---- phase 2: attention, lag-2 pipelined; softmax denom off-PE
            with (
                tc.tile_pool(name="acc", bufs=1, space="PSUM") as acc,
                tc.tile_pool(name="pds", bufs=2, space="PSUM") as pds,
                tc.tile_pool(name="attnp", bufs=3) as attnp,
                tc.tile_pool(name="qsp", bufs=3) as qsp,
                tc.tile_pool(name="vtp", bufs=3) as vtp,
                tc.tile_pool(name="sm1", bufs=1) as sm1,
                tc.tile_pool(name="sm2", bufs=2) as sm2,
            ):
                o2ps = [
                    acc.tile([P, IQ], F32, tag=f"o2_{s}", name=f"o2_{s}")
                    for s in range(NC6)
                ]
                s1 = sm1.tile([P, 4, IQ], BF16)
                s2 = sm1.tile([P, 2, IQ], BF16)

                def make_qs(g):
                    # qs(g) on ACT: per-partition scale fold. Emitted two
                    # iterations ahead so ACT never blocks the dots stream.
                    qs = qsp.tile([P, NC6, IQ], BF16, tag="qs", name=f"qs{g}")
                    for c in range(NC6):
                        nc.scalar.mul(
                            qs[:, c, :], qT[:, c, :],
                            scaleT[:, c * H + g : c * H + g + 1],
                        )
                    return qs

                qss = {0: make_qs(0), 1: make_qs(1)}
                attnTs, vts = {}, {}
                for it in range(H + 2):
                    if it == 1:
                        # Wout is only read by phase 3; issuing its DMA here
                        # keeps it out of the prologue bandwidth window.
                        nc.gpsimd.dma_start(Wout_sb[:], r3(Wout))
                    if it < H:
                        g = it
                        qs = qss.pop(g)
                        # Vt(g) on DVE: mix_post column fold per 64-col group
                        vt = vtp.tile([P, JC8, DIM], BF16, tag="vt")
                        vts[g] = vt
                        for gp in range(H):
                            nc.vector.tensor_scalar_mul(
                                vt[:, :, gp * DH : (gp + 1) * DH],
                                V[:, :, gp * DH : (gp + 1) * DH],
                                m2bc[:, g * H + gp : g * H + gp + 1],
                            )
                        # dots(g) on PE + exp on ACT
                        attnT = attnp.tile([P, JC8, IQ], BF16, tag="attnT")
                        attnTs[g] = attnT
                        for jc in range(JC8):
                            ds = pds.tile([P, IQ], F32, tag="ds")
                            for c in range(NC6):
                                nc.tensor.matmul(
                                    ds[:], kT[:, c, jc * P : (jc + 1) * P],
                                    qs[:, c, :],
                                    start=(c == 0), stop=(c == NC6 - 1),
                                )
                            nc.scalar.activation(attnT[:, jc, :], ds[:], EXP)
                        # softmax denominator: DVE jc-tree + GPSIMD allreduce
                        Ssum = sm2.tile([P, IQ], F32, tag="Ssum")
                        rR = sm2.tile([P, IQ], F32, tag="rR")
                        rRb = sm2.tile([P, IQ], BF16, tag="rRb")
                        nc.vector.tensor_tensor(
                            s1[:], attnT[:, 0:4, :], attnT[:, 4:8, :], ADD
                        )
                        nc.vector.tensor_tensor(
                            s2[:], s1[:, 0:2, :], s1[:, 2:4, :], ADD
                        )
                        nc.vector.tensor_tensor(
                            Ssum[:], s2[:, 0, :], s2[:, 1, :], ADD
                        )
                        nc.gpsimd.partition_all_reduce(
                            Ssum[:], Ssum[:], channels=P,
                            reduce_op=bass_isa.ReduceOp.add,
                        )
                        nc.vector.reciprocal_approx_fast(rR[:], Ssum[:])
                        nc.scalar.copy(rRb[:], rR[:])
                        nc.vector.tensor_tensor(
                            attnT[:], attnT[:],
                            rRb[:, None, :].to_broadcast((P, JC8, IQ)), MULT,
                        )
                        if g + 2 < H:
                            qss[g + 2] = make_qs(g + 2)
                    if it >= 2:
                        g2 = it - 2
                        for s in range(NC6):
                            for jc in range(JC8):
                                nc.tensor.matmul(
                                    o2ps[s][:],
                                    vts[g2][:, jc, s * P : (s + 1) * P],
                                    attnTs[g2][:, jc, :],
                                    start=(g2 == 0 and jc == 0),
                                    stop=(g2 == H - 1 and jc == JC8 - 1),
                                )
                        del attnTs[g2], vts[g2]

                for s in range(NC6):
                    if s % 2 == 0:
                        nc.vector.tensor_copy(o2_sb[:, s, :], o2ps[s][:])
                    else:
                        nc.scalar.copy(o2_sb[:, s, :], o2ps[s][:])

            # ---- phase 3: output projection (f32r) + bias via K=1 matmul ----
            with (
                tc.tile_pool(name="pj3", bufs=2, space="PSUM") as pj3,
                tc.tile_pool(name="ob", bufs=2) as ob,
            ):
                for isl in range(IQ // P):
                    fp = pj3.tile([P, DIM], F32, tag="fin")
                    for ns, ne in ((0, 512), (512, DIM)):
                        for c in range(NC6):
                            nc.tensor.matmul(
                                fp[:, ns:ne],
                                o2_sb[:, c, isl * P : (isl + 1) * P],
                                Wout_sb[:, c, ns:ne],
                                start=(c == 0), stop=False,
                            )
                        nc.tensor.matmul(
                            fp[:, ns:ne],
                            ones_row[:],
                            bout_sb[:, ns:ne],
                            start=False, stop=True,
                        )
                    osb = ob.tile([P, DIM], F32, tag="osb")
                    nc.scalar.copy(osb[:], fp[:])
                    nc.gpsimd.dma_start(out[isl * P : (isl + 1) * P, :], osb[:])

    nc.compile()
    return nc


def kernel(x, Wq, Wkv, mix_pre, mix_post, Wout, bout):
    x = np.asarray(x, dtype=np.float32)
    Wq = np.asarray(Wq, dtype=np.float32)
    Wkv = np.asarray(Wkv, dtype=np.float32)
    mix_pre = np.asarray(mix_pre, dtype=np.float32)
    mix_post = np.asarray(mix_post, dtype=np.float32)
    Wout = np.asarray(Wout, dtype=np.float32)
    bout = np.asarray(bout, dtype=np.float32)

    if "nc" not in _CACHE:
        _CACHE["nc"] = _build_nc()
    nc = _CACHE["nc"]

    import ml_dtypes
    bf = ml_dtypes.bfloat16
    Wk = np.ascontiguousarray(Wkv[:, :DIM]).astype(bf)
    Wv = np.ascontiguousarray(Wkv[:, DIM:]).astype(bf)
    # scaleT[p, c*H+g] = SCALE * mix_pre[(c*128+p)//64, g]
    SM = SCALE * mix_pre[np.arange(DIM) // DH, :]          # [768, 12]
    scaleT = np.ascontiguousarray(
        SM.reshape(NC6, P, H).transpose(1, 0, 2).reshape(P, NC6 * H)
    ).astype(np.float32)
    shared = {
        "Wq": Wq.astype(bf), "Wk": Wk, "Wv": Wv, "Wout": Wout.astype(bf),
        "bout": np.ascontiguousarray(bout.reshape(1, DIM)).astype(bf),
        "scaleT": scaleT,
        "m2": np.ascontiguousarray(mix_post.reshape(1, H * H)),
    }
    b_, n_, d_ = x.shape
    in_maps = []
    for c in range(8):
        b, half = c // 2, c % 2
        m = dict(shared)
        m["xqT"] = np.ascontiguousarray(x[b, half * IQ : (half + 1) * IQ, :].T).astype(bf)
        m["xkvT"] = np.ascontiguousarray(x[b].T).astype(bf)
        in_maps.append(m)

    res = run_bass_kernel_spmd(nc, in_maps, core_ids=list(range(8)))
    _CACHE["last_results"] = res
    _CACHE["last_in_maps"] = in_maps

    full = np.empty((b_, n_, d_), dtype=np.float32)
    for c in range(8):
        b, half = c // 2, c % 2
        full[b, half * IQ : (half + 1) * IQ, :] = res.results[c]["out"]
    return full


# revision 20
# speedup vs baseline: 1.1866x; 1.0183x over previous
"""Talking-heads attention on 8 Trainium2 NeuronCores.

Sharding: data-parallel over (batch b in 0..3) x (query half in 0..1) -> 8 cores.
Each core computes K/V for its full batch sequence (1024) and attention for its
512 query rows. No collectives.

Math notes (per core, all layouts transposed so contractions sit on partitions):
  - mix_pre (and SCALE) folded into Q per output-head g:
    qs_g[hd, i] = qT[hd, i] * scaleT[hd, g], scaleT = SCALE*mix_pre[h(hd), g]
    (host-precomputed), so dotsT_g[j, i] = sum_hd kT[hd, j] * qs_g[hd, i].
  - softmax over j (partitions) without max-subtraction (|dots| <~ 6, safe).
    The denominator never touches the PE: DVE tree-sums the 8 key chunks,
    GPSIMD partition_all_reduce sums across partitions (result broadcast to
    all partitions), DVE reciprocal + in-place scale of attnT.
  - mix_post folded into V: Vt_g[j, (g',d)] = mix_post[g, g'] * v[j, (g',d)];
    out2T[(g'd), i] += sum_j Vt_g[j, gd] * attnT_g[j, i] accumulated in PSUM
    over g (6 banks) while dots for g+2 stream (lag-2 software pipeline).
  - out = out2T.T @ Wout + bout, f32r matmuls (1 cyc/row), bias folded in as
    a K=1 accumulating matmul.
"""

import numpy as np

import concourse.bass as bass
import concourse.bass_isa as bass_isa
import concourse.mybir as mybir
import concourse.tile as tile
from concourse import bacc
from concourse.bass_utils import run_bass_kernel_spmd

P = 128
DIM = 768
SEQ = 1024
IQ = 512            # query rows per core
H = 12
DH = 64
NC6 = DIM // P      # 6 chunks of the 768 dim
JC8 = SEQ // P      # 8 chunks of the key dim
SCALE = DH ** -0.5
F32 = mybir.dt.float32
F32R = mybir.dt.float32r
BF16 = mybir.dt.bfloat16
EXP = mybir.ActivationFunctionType.Exp
ADD = mybir.AluOpType.add
MULT = mybir.AluOpType.mult

_CACHE = {}


def _build_nc():
    nc = bacc.Bacc("TRN2", target_bir_lowering=False, debug=False)

    xqT = nc.dram_tensor("xqT", [DIM, IQ], BF16, kind="ExternalInput")
    xkvT = nc.dram_tensor("xkvT", [DIM, SEQ], BF16, kind="ExternalInput")
    Wq = nc.dram_tensor("Wq", [DIM, DIM], BF16, kind="ExternalInput")
    Wk = nc.dram_tensor("Wk", [DIM, DIM], BF16, kind="ExternalInput")
    Wv = nc.dram_tensor("Wv", [DIM, DIM], BF16, kind="ExternalInput")
    Wout = nc.dram_tensor("Wout", [DIM, DIM], BF16, kind="ExternalInput")
    bout = nc.dram_tensor("bout", [1, DIM], BF16, kind="ExternalInput")
    scaleT_d = nc.dram_tensor("scaleT", [P, NC6 * H], F32, kind="ExternalInput")
    m2_d = nc.dram_tensor("m2", [1, H * H], F32, kind="ExternalInput")
    out = nc.dram_tensor("out", [IQ, DIM], F32, kind="ExternalOutput")

    r3 = lambda t: t.rearrange("(c p) e -> p c e", p=P)

    with tile.TileContext(nc) as tc:
        with tc.tile_pool(name="persist", bufs=1) as pp:
            # ---- persistent tiles ----
            qT = pp.tile([P, NC6, IQ], BF16)
            kT = pp.tile([P, NC6, SEQ], BF16)
            V = pp.tile([P, JC8, DIM], BF16)     # [j-part, jc, (g,d)]
            Wout_sb = pp.tile([P, NC6, DIM], BF16)
            o2_sb = pp.tile([P, NC6, IQ], BF16)  # out2T staged for out-proj
            scaleT = pp.tile([P, NC6 * H], F32)  # SCALE*mix_pre expanded
            q01 = pp.tile([P, 2, NC6, IQ], BF16)  # qs for g=0,1, made in phase 1
            m2_sb = pp.tile([1, H * H], F32)
            m2bc = pp.tile([P, H * H], F32)      # mix_post bcast to all parts
            bout_sb = pp.tile([1, DIM], BF16)
            ones_row = pp.tile([1, P], BF16)
            nc.gpsimd.memset(ones_row[:], 1.0)

            # ---- phase 1: input DMA (priority-ordered, chunked) + projections
            with (
                tc.tile_pool(name="pin", bufs=1) as pin,
                tc.tile_pool(name="pj", bufs=2, space="PSUM") as pj,
                tc.tile_pool(name="pjv", bufs=2, space="PSUM") as pjv,
            ):
                xqT_sb = pin.tile([P, NC6, IQ], BF16)
                Wq_sb = pin.tile([P, NC6, DIM], BF16)
                xkvT_sb = pin.tile([P, NC6, SEQ], BF16)
                Wk_sb = pin.tile([P, NC6, DIM], BF16)
                Wv_sb = pin.tile([P, NC6, DIM], BF16)
                # input DMA: one queue, one descriptor per tensor, in
                # consumption order — transfers pipeline behind the NEFF
                # startup window and the q projection.
                nc.sync.dma_start(Wq_sb[:], r3(Wq))
                nc.sync.dma_start(xqT_sb[:], r3(xqT))
                nc.sync.dma_start(xkvT_sb[:], r3(xkvT))
                nc.sync.dma_start(Wk_sb[:], r3(Wk))
                nc.sync.dma_start(Wv_sb[:], r3(Wv))
                nc.gpsimd.dma_start(scaleT[:], scaleT_d[:])
                nc.gpsimd.dma_start(m2_sb[:], m2_d[:])
                nc.gpsimd.dma_start(bout_sb[:], bout[:])
                nc.gpsimd.partition_broadcast(m2bc[:], m2_sb[:])

                # qT[e,i] = sum_f Wq[f,e] xqT[f,i]
                for ec in range(NC6):
                    ps = pj.tile([P, IQ], F32, tag="pjq")
                    for fc in range(NC6):
                        nc.tensor.matmul(
                            ps[:], Wq_sb[:, fc, ec * P : (ec + 1) * P],
                            xqT_sb[:, fc, :], start=(fc == 0), stop=(fc == NC6 - 1),
                        )
                    nc.scalar.copy(qT[:, ec, :], ps[:])

                # qs for g=0,1 while ACT is otherwise idle (keeps the g-loop
                # fill iterations free of qs work)
                for g01 in range(2):
                    for c in range(NC6):
                        nc.scalar.mul(
                            q01[:, g01, c, :], qT[:, c, :],
                            scaleT[:, c * H + g01 : c * H + g01 + 1],
                        )

                # kT[e,j]
                for ec in range(NC6):
                    for jh in range(2):
                        ps = pj.tile([P, IQ], F32, tag="pjq")
                        for fc in range(NC6):
                            nc.tensor.matmul(
                                ps[:], Wk_sb[:, fc, ec * P : (ec + 1) * P],
                                xkvT_sb[:, fc, jh * IQ : (jh + 1) * IQ],
                                start=(fc == 0), stop=(fc == NC6 - 1),
                            )
                        nc.scalar.copy(kT[:, ec, jh * IQ : (jh + 1) * IQ], ps[:])

                # V[j, gd] = sum_f xkvT[f, j] Wv[f, gd]
                for jc in range(JC8):
                    psv = pjv.tile([P, DIM], F32, tag="pjv")
                    for ns, ne in ((0, 512), (512, DIM)):
                        for fc in range(NC6):
                            nc.tensor.matmul(
                                psv[:, ns:ne],
                                xkvT_sb[:, fc, jc * P : (jc + 1) * P],
                                Wv_sb[:, fc, ns:ne],
                                start=(fc == 0), stop=(fc == NC6 - 1),
                            )
                    nc.scalar.copy(V[:, jc, :], psv[:])

            # ---- phase 2: attention, lag-2 pipelined; softmax denom off-PE
            with (
                tc.tile_pool(name="acc", bufs=1, space="PSUM") as acc,
                tc.tile_pool(name="pds", bufs=2, space="PSUM") as pds,
                tc.tile_pool(name="attnp", bufs=3) as attnp,
                tc.tile_pool(name="qsp", bufs=3) as qsp,
                tc.tile_pool(name="vtp", bufs=3) as vtp,
                tc.tile_pool(name="sm1", bufs=1) as sm1,
                tc.tile_pool(name="sm2", bufs=2) as sm2,
            ):
                o2ps = [
                    acc.tile([P, IQ], F32, tag=f"o2_{s}", name=f"o2_{s}")
                    for s in range(NC6)
                ]
                s1 = sm1.tile([P, 4, IQ], BF16)
                s2 = sm1.tile([P, 2, IQ], BF16)

                def make_qs(g):
                    # qs(g) on ACT: per-partition scale fold. Emitted two
                    # iterations ahead so ACT never blocks the dots stream.
                    qs = qsp.tile([P, NC6, IQ], BF16, tag="qs", name=f"qs{g}")
                    for c in range(NC6):
                        nc.scalar.mul(
                            qs[:, c, :], qT[:, c, :],
                            scaleT[:, c * H + g : c * H + g + 1],
                        )
                    return qs

                qss = {0: q01[:, 0], 1: q01[:, 1]}
                attnTs, vts = {}, {}
                for it in range(H + 2):
                    if it == 1:
                        # Wout is only read by phase 3; issuing its DMA here
                        # keeps it out of the prologue bandwidth window.
                        nc.gpsimd.dma_start(Wout_sb[:], r3(Wout))
                    if it < H:
                        g = it
                        qs = qss.pop(g)
                        # Vt(g) on DVE: mix_post column fold per 64-col group
                        vt = vtp.tile([P, JC8, DIM], BF16, tag="vt")
                        vts[g] = vt
                        for gp in range(H):
                            nc.vector.tensor_scalar_mul(
                                vt[:, :, gp * DH : (gp + 1) * DH],
                                V[:, :, gp * DH : (gp + 1) * DH],
                                m2bc[:, g * H + gp : g * H + gp + 1],
                            )
                        # dots(g) on PE + exp on ACT
                        attnT = attnp.tile([P, JC8, IQ], BF16, tag="attnT")
                        attnTs[g] = attnT
                        for jc in range(JC8):
                            ds = pds.tile([P, IQ], F32, tag="ds")
                            for c in range(NC6):
                                nc.tensor.matmul(
                                    ds[:], kT[:, c, jc * P : (jc + 1) * P],
                                    qs[:, c, :],
                                    start=(c == 0), stop=(c == NC6 - 1),
                                )
                            nc.scalar.activation(attnT[:, jc, :], ds[:], EXP)
                        # softmax denominator: DVE jc-tree + GPSIMD allreduce
                        Ssum = sm2.tile([P, IQ], F32, tag="Ssum")
                        rR = sm2.tile([P, IQ], F32, tag="rR")
                        rRb = sm2.tile([P, IQ], BF16, tag="rRb")
                        nc.vector.tensor_tensor(
                            s1[:], attnT[:, 0:4, :], attnT[:, 4:8, :], ADD
                        )
                        nc.vector.tensor_tensor(
                            s2[:], s1[:, 0:2, :], s1[:, 2:4, :], ADD
                        )
                        nc.vector.tensor_tensor(
                            Ssum[:], s2[:, 0, :], s2[:, 1, :], ADD
                        )
                        nc.gpsimd.partition_all_reduce(
                            Ssum[:], Ssum[:], channels=P,
                            reduce_op=bass_isa.ReduceOp.add,
                        )
                        nc.vector.reciprocal_approx_fast(rR[:], Ssum[:])
                        nc.scalar.copy(rRb[:], rR[:])
                        nc.vector.tensor_tensor(
                            attnT[:], attnT[:],
                            rRb[:, None, :].to_broadcast((P, JC8, IQ)), MULT,
                        )
                        if g + 2 < H:
                            qss[g + 2] = make_qs(g + 2)
                    if it >= 2:
                        g2 = it - 2
                        for s in range(NC6):
                            for jc in range(JC8):
                                nc.tensor.matmul(
                                    o2ps[s][:],
                                    vts[g2][:, jc, s * P : (s + 1) * P],
                                    attnTs[g2][:, jc, :],
                                    start=(g2 == 0 and jc == 0),
                                    stop=(g2 == H - 1 and jc == JC8 - 1),
                                )
                        del attnTs[g2], vts[g2]

                for s in range(NC6):
                    if s % 2 == 0:
                        nc.vector.tensor_copy(o2_sb[:, s, :], o2ps[s][:])
                    else:
                        nc.scalar.copy(o2_sb[:, s, :], o2ps[s][:])

            # ---- phase 3: output projection (f32r) + bias via K=1 matmul ----
            with (
                tc.tile_pool(name="pj3", bufs=2, space="PSUM") as pj3,
                tc.tile_pool(name="ob", bufs=2) as ob,
            ):
                for isl in range(IQ // P):
                    fp = pj3.tile([P, DIM], F32, tag="fin")
                    for ns, ne in ((0, 512), (512, DIM)):
                        for c in range(NC6):
                            nc.tensor.matmul(
                                fp[:, ns:ne],
                                o2_sb[:, c, isl * P : (isl + 1) * P],
                                Wout_sb[:, c, ns:ne],
                                start=(c == 0), stop=False,
                            )
                        nc.tensor.matmul(
                            fp[:, ns:ne],
                            ones_row[:],
                            bout_sb[:, ns:ne],
                            start=False, stop=True,
                        )
                    osb = ob.tile([P, DIM], F32, tag="osb")
                    nc.scalar.copy(osb[:], fp[:])
                    nc.gpsimd.dma_start(out[isl * P : (isl + 1) * P, :], osb[:])

    nc.compile()
    return nc


def kernel(x, Wq, Wkv, mix_pre, mix_post, Wout, bout):
    x = np.asarray(x, dtype=np.float32)
    Wq = np.asarray(Wq, dtype=np.float32)
    Wkv = np.asarray(Wkv, dtype=np.float32)
    mix_pre = np.asarray(mix_pre, dtype=np.float32)
    mix_post = np.asarray(mix_post, dtype=np.float32)
    Wout = np.asarray(Wout, dtype=np.float32)
    bout = np.asarray(bout, dtype=np.float32)

    if "nc" not in _CACHE:
        _CACHE["nc"] = _build_nc()
    nc = _CACHE["nc"]

    import ml_dtypes
    bf = ml_dtypes.bfloat16
    Wk = np.ascontiguousarray(Wkv[:, :DIM]).astype(bf)
    Wv = np.ascontiguousarray(Wkv[:, DIM:]).astype(bf)
    # scaleT[p, c*H+g] = SCALE * mix_pre[(c*128+p)//64, g]
    SM = SCALE * mix_pre[np.arange(DIM) // DH, :]          # [768, 12]
    scaleT = np.ascontiguousarray(
        SM.reshape(NC6, P, H).transpose(1, 0, 2).reshape(P, NC6 * H)
    ).astype(np.float32)
    shared = {
        "Wq": Wq.astype(bf), "Wk": Wk, "Wv": Wv, "Wout": Wout.astype(bf),
        "bout": np.ascontiguousarray(bout.reshape(1, DIM)).astype(bf),
        "scaleT": scaleT,
        "m2": np.ascontiguousarray(mix_post.reshape(1, H * H)),
    }
    b_, n_, d_ = x.shape
    in_maps = []
    for c in range(8):
        b, half = c // 2, c % 2
        m = dict(shared)
        m["xqT"] = np.ascontiguousarray(x[b, half * IQ : (half + 1) * IQ, :].T).astype(bf)
        m["xkvT"] = np.ascontiguousarray(x[b].T).astype(bf)
        in_maps.append(m)

    res = run_bass_kernel_spmd(nc, in_maps, core_ids=list(range(8)))
    _CACHE["last_results"] = res
    _CACHE["last_in_maps"] = in_maps

    full = np.empty((b_, n_, d_), dtype=np.float32)
    for c in range(8):
        b, half = c // 2, c % 2
        full[b, half * IQ : (half + 1) * IQ, :] = res.results[c]["out"]
    return full
